# revision 1
# baseline (speedup 1.0000x reference)
"""Cost-volume kernel for Trainium2 (Bass/Tile), 8-core SPMD.

Problem: left/right features [B=2, C=32, H=128, W=256] f32.
Output [B, 2C=64, D=48, H, W] where for disparity d in [-8, 40):
  out[:, 0:C,  d+8, h, x] = left[:, :, h, x]   if 0 <= x-d < W else 0
  out[:, C:2C, d+8, h, x] = right[:, :, h, x-d] if 0 <= x-d < W else 0

Sharding: channels split 4-per-core (8 cores, identical program).
Each core builds the full disparity band for its 4 left + 4 right
channels. Pure data-movement kernel, bound by the HBM write rate of
the 96 MiB/core output.

Perf notes (from NTFF traces):
  - HWDGE (sync/scalar) DMA rings only engage 8 of the 16 SDMA
    engines; SWDGE (gpsimd) engages all 16. All big transfers go SWDGE.
  - Every store is a full-width DMA with contiguous 8 KiB/partition
    source rows (128 descriptors of 8 KiB), which sustains near line
    rate. Right-side shifted windows are materialized by DVE copies
    into contiguous staging buffers to keep descriptors at 8 KiB.
  - Zero padding is produced in SBUF (host-padded right image, SBUF
    memsets for left), never as thin strided DRAM writes.
  - The right input arrives host-padded so no SBUF memset gates the
    first staging copies; left-buffer prep is emitted lazily to keep
    the gpsimd DMA FIFO from head-of-line blocking at startup.
"""

import numpy as np

B, C, H, W = 2, 32, 128, 256
MIN_D, MAX_D = -8, 40
D = MAX_D - MIN_D  # 48
N_CORES = 8
CPC = C // N_CORES  # 4 channels of each image per core
BC = B * CPC  # 8 (b, c) pairs per core

PAD_L = 39  # covers max shift d=39 (offset = x - d + PAD_L >= 0)
PAD_R = 9   # covers min shift d=-8 (x - d <= 263 -> offset 302 < 304)
WP = PAD_L + W + PAD_R  # 304

HL = 8            # h rows held per partition
HH = H // HL      # 16
NPART = BC * HH   # 128 partitions: p = (b*CPC + c)*HH + h_hi

POS_BUFS = 4  # left work buffers for d >= 0 (buffer j: d = j, j+4, ... asc)
NEG_BUFS = 2  # left work buffers for d < 0 (buffer j: d = -(j+1), -(j+1)-2, ... desc)
STAGE_BUFS = 16  # right staging rotation depth (deep: keeps SDMA queues fed)

# Disabled (= MAX_D): writing only the valid columns [d, W) for large d
# and leaving the zero band to the runtime's pre-zeroed output buffers
# was measured SLOWER (348 us vs 298 us) — sub-1-KiB DRAM runs cost more
# in SWDGE descriptor/packet overhead than the ~2 MB of zeros they save.
DIRECT_D = MAX_D

# store order for the left side: negatives interleaved early; within a
# buffer positives ascend and negatives descend (zero regions only grow).
LEFT_ORDER = [0, -1, 1, -2, 2, 3, -3, 4, 5, -4, 6, 7, -5, 8, 9, -6, 10,
              11, -7, 12, 13, -8] + list(range(14, MAX_D))
assert sorted(LEFT_ORDER) == list(range(MIN_D, MAX_D))

_CACHE = {}


def _build_nc():
    import concourse.bacc as bacc
    import concourse.tile as tile
    import concourse.mybir as mybir

    f32 = mybir.dt.float32
    nc = bacc.Bacc(
        "TRN2",
        target_bir_lowering=False,
        debug=False,
        enable_asserts=False,
        num_devices=N_CORES,
    )
    left_in = nc.dram_tensor("left_in", [B, CPC, H, W], f32, kind="ExternalInput")
    right_in = nc.dram_tensor(
        "right_in", [B, CPC, H, WP], f32, kind="ExternalInput"
    )  # host-padded with zeros: data columns at [PAD_L, PAD_L + W)
    left_out = nc.dram_tensor(
        "left_out", [B, CPC, D, H, W], f32, kind="ExternalOutput"
    )
    right_out = nc.dram_tensor(
        "right_out", [B, CPC, D, H, W], f32, kind="ExternalOutput"
    )

    with tile.TileContext(nc) as tc:
        with (
            tc.tile_pool(name="pool", bufs=1) as pool,
            tc.tile_pool(name="stpool", bufs=STAGE_BUFS) as stpool,
        ):
            # ---- right image (pre-padded), loaded once ----
            rp = pool.tile([NPART, HL * WP], f32, tag="rp")
            rp3 = rp[:].rearrange("p (h w) -> p h w", h=HL)
            # zero source for left-edge zeroing, done as ACT copies so the
            # WAR-gated zeroing never head-of-line blocks the in-order DVE
            # queue that feeds the right-side staging copies
            zt = pool.tile([NPART, HL * max(POS_BUFS, NEG_BUFS)], f32, tag="zt")
            zt3 = zt[:].rearrange("p (h w) -> p h w", h=HL)
            nc.vector.memset(zt[:], 0.0)

            def zero_cols(t3, a, b):
                nc.scalar.copy(t3[:, :, a:b], zt3[:, :, 0 : b - a])

            # ---- left work buffers; pos[0] is the load target, the rest
            # are DVE-copied lazily on first use ----
            pos = []
            neg = []
            for j in range(POS_BUFS):
                t = pool.tile([NPART, HL * W], f32, tag=f"lp{j}")
                pos.append([t, t[:].rearrange("p (h w) -> p h w", h=HL), False])
            for j in range(NEG_BUFS):
                t = pool.tile([NPART, HL * W], f32, tag=f"ln{j}")
                neg.append([t, t[:].rearrange("p (h w) -> p h w", h=HL), False])
            pos[0][2] = True  # loaded directly, no copy needed
            # left load first: the d=0 left store depends only on it, so it
            # is the earliest possible store; right stores additionally
            # need a DVE staging copy after the right load lands
            nc.gpsimd.dma_start(pos[0][0][:], left_in.ap())
            nc.gpsimd.dma_start(rp[:], right_in.ap())

            def emit_left(d):
                if d >= DIRECT_D:
                    # valid columns only; zero band stays pre-zeroed DRAM.
                    # pos[0]'s work-buffer cycle only zeroes columns
                    # [0, DIRECT_D - POS_BUFS), disjoint from [d, W).
                    nc.gpsimd.dma_start(
                        left_out.ap()[:, :, d - MIN_D, :, d:W],
                        pos[0][1][:, :, d:W],
                    )
                    return
                if d >= 0:
                    buf = pos[d % POS_BUFS]
                    assert d < DIRECT_D
                    t, t3, ready = buf
                    if not ready:
                        nc.scalar.copy(t[:], pos[0][0][:])
                        if d > 0:
                            zero_cols(t3, 0, d)
                        buf[2] = True
                    elif d >= POS_BUFS:
                        zero_cols(t3, d - POS_BUFS, d)
                else:
                    buf = neg[(-d - 1) % NEG_BUFS]
                    t, t3, ready = buf
                    if not ready:
                        nc.scalar.copy(t[:], pos[0][0][:])
                        zero_cols(t3, W + d, W)
                        buf[2] = True
                    else:
                        zero_cols(t3, W + d, W + d + NEG_BUFS)
                nc.gpsimd.dma_start(left_out.ap()[:, :, d - MIN_D, :, :], t[:])

            def emit_right(di):
                d = di + MIN_D
                a = PAD_L - d
                stage = stpool.tile([NPART, HL * W], f32, tag="st")
                st3 = stage[:].rearrange("p (h w) -> p h w", h=HL)
                nc.vector.tensor_copy(st3[:], rp3[:, :, a : a + W])
                nc.gpsimd.dma_start(right_out.ap()[:, :, di, :, :], stage[:])

            emit_left(LEFT_ORDER[0])
            emit_left(LEFT_ORDER[1])
            for step in range(D):
                emit_right(step)
                if step + 2 < D:
                    emit_left(LEFT_ORDER[step + 2])

    nc.compile()
    return nc


def _get_nc():
    if "nc" not in _CACHE:
        _CACHE["nc"] = _build_nc()
    return _CACHE["nc"]


def kernel(left_feat, right_feat):
    from concourse.bass_utils import run_bass_kernel_spmd

    left = np.ascontiguousarray(np.asarray(left_feat), dtype=np.float32)
    right = np.ascontiguousarray(np.asarray(right_feat), dtype=np.float32)
    assert left.shape == (B, C, H, W) and right.shape == (B, C, H, W)

    nc = _get_nc()
    right_pad = np.zeros((B, C, H, WP), dtype=np.float32)
    right_pad[:, :, :, PAD_L : PAD_L + W] = right
    in_maps = []
    for m in range(N_CORES):
        sl = slice(m * CPC, (m + 1) * CPC)
        in_maps.append(
            {
                "left_in": np.ascontiguousarray(left[:, sl]),
                "right_in": np.ascontiguousarray(right_pad[:, sl]),
            }
        )
    res = run_bass_kernel_spmd(nc, in_maps, core_ids=list(range(N_CORES))).results

    out = np.empty((B, 2 * C, D, H, W), dtype=np.float32)
    for m in range(N_CORES):
        sl = slice(m * CPC, (m + 1) * CPC)
        out[:, sl] = res[m]["left_out"]
        out[:, C + m * CPC : C + (m + 1) * CPC] = res[m]["right_out"]
    return out



# revision 2
# speedup vs baseline: 1.7933x; 1.7933x over previous
"""Cost-volume kernel for Trainium2 (Bass/Tile), 8-core SPMD.

Problem: left/right features [B=2, C=32, H=128, W=256] f32.
Output [B, 2C=64, D=48, H, W] where for disparity d in [-8, 40):
  out[:, 0:C,  d+8, h, x] = left[:, :, h, x]   if 0 <= x-d < W else 0
  out[:, C:2C, d+8, h, x] = right[:, :, h, x-d] if 0 <= x-d < W else 0

This is a pure data-movement kernel bound by HBM write bandwidth
(~358 GB/s per core). Two levers vs the f32 baseline (298 us):
  - fp16 end-to-end: host quantizes inputs to fp16, the device moves
    fp16 (half the HBM bytes), host upcasts the output to f32. The
    quantization rel-err (~5e-4) is far inside the 2e-2 gate.
  - H-row sharding (16 rows of H per core) instead of channel
    sharding: per-core input reads drop 2x (each core reads only its
    row band of both images).

Sharding: H split 16-rows-per-core (8 cores, identical program).
Each core builds the full disparity volume for all 64 channels of its
row band. Per-core HBM traffic: 48 MiB out + ~1.1 MiB in.

Perf notes inherited from the f32 baseline (NTFF traces):
  - SWDGE (gpsimd) engages all 16 SDMA engines; HWDGE only 8. All
    big transfers go SWDGE.
  - Every store is a full-width DMA with contiguous 4 KiB/partition
    source rows. Right-side shifted windows are materialized by DVE
    copies into contiguous staging buffers.
  - Zero padding is produced in SBUF (host-padded right image, SBUF
    memsets for left), never as thin strided DRAM writes (measured
    slower at f32: 348 vs 298 us).
  - Left-buffer prep runs on ACT so WAR-gated zeroing never blocks
    the in-order DVE queue feeding the right staging copies.
"""

import numpy as np

B, C, H, W = 2, 32, 128, 256
MIN_D, MAX_D = -8, 40
D = MAX_D - MIN_D  # 48
N_CORES = 8
HB = H // N_CORES  # 16 rows of H per core

PAD_L = 39  # covers max shift d=39 (offset = x - d + PAD_L >= 0)
PAD_R = 9   # covers min shift d=-8 (x - d <= 263 -> offset 302 < 304)
WP = PAD_L + W + PAD_R  # 304

HL = 8             # h rows held per partition
HH = HB // HL      # 2
NPART = B * C * HH  # 128 partitions: p = (b*C + c)*HH + h_hi

POS_BUFS = 4  # left work buffers for d >= 0 (buffer j: d = j, j+4, ... asc)
NEG_BUFS = 2  # left work buffers for d < 0 (buffer j: d = -(j+1), -(j+1)-2, ... desc)
STAGE_BUFS = 16  # right staging rotation depth (deep: keeps SDMA queues fed)

# store order for the left side: negatives interleaved early; within a
# buffer positives ascend and negatives descend (zero regions only grow).
LEFT_ORDER = [0, -1, 1, -2, 2, 3, -3, 4, 5, -4, 6, 7, -5, 8, 9, -6, 10,
              11, -7, 12, 13, -8] + list(range(14, MAX_D))
assert sorted(LEFT_ORDER) == list(range(MIN_D, MAX_D))

_CACHE = {}


def _build_nc():
    import concourse.bacc as bacc
    import concourse.tile as tile
    import concourse.mybir as mybir

    f16 = mybir.dt.float16
    nc = bacc.Bacc(
        "TRN2",
        target_bir_lowering=False,
        debug=False,
        enable_asserts=False,
        num_devices=N_CORES,
    )
    left_in = nc.dram_tensor("left_in", [B, C, HB, W], f16, kind="ExternalInput")
    right_in = nc.dram_tensor(
        "right_in", [B, C, HB, WP], f16, kind="ExternalInput"
    )  # host-padded with zeros: data columns at [PAD_L, PAD_L + W)
    left_out = nc.dram_tensor(
        "left_out", [B, C, D, HB, W], f16, kind="ExternalOutput"
    )
    right_out = nc.dram_tensor(
        "right_out", [B, C, D, HB, W], f16, kind="ExternalOutput"
    )

    with tile.TileContext(nc) as tc:
        with (
            tc.tile_pool(name="pool", bufs=1) as pool,
            tc.tile_pool(name="stpool", bufs=STAGE_BUFS) as stpool,
        ):
            # ---- right image (pre-padded), loaded once ----
            rp = pool.tile([NPART, HL * WP], f16, tag="rp")
            rp3 = rp[:].rearrange("p (h w) -> p h w", h=HL)
            # zero source for left-edge zeroing, done as ACT copies so the
            # WAR-gated zeroing never head-of-line blocks the in-order DVE
            # queue that feeds the right-side staging copies
            zt = pool.tile([NPART, HL * max(POS_BUFS, NEG_BUFS)], f16, tag="zt")
            zt3 = zt[:].rearrange("p (h w) -> p h w", h=HL)
            nc.vector.memset(zt[:], 0.0)

            def zero_cols(t3, a, b):
                nc.scalar.copy(t3[:, :, a:b], zt3[:, :, 0 : b - a])

            # ---- left work buffers; pos[0] is the load target, the rest
            # are copied lazily on first use ----
            pos = []
            neg = []
            for j in range(POS_BUFS):
                t = pool.tile([NPART, HL * W], f16, tag=f"lp{j}")
                pos.append([t, t[:].rearrange("p (h w) -> p h w", h=HL), False])
            for j in range(NEG_BUFS):
                t = pool.tile([NPART, HL * W], f16, tag=f"ln{j}")
                neg.append([t, t[:].rearrange("p (h w) -> p h w", h=HL), False])
            pos[0][2] = True  # loaded directly, no copy needed
            # left load first: the d=0 left store depends only on it, so it
            # is the earliest possible store; right stores additionally
            # need a DVE staging copy after the right load lands
            nc.gpsimd.dma_start(pos[0][0][:], left_in.ap())
            nc.gpsimd.dma_start(rp[:], right_in.ap())

            def emit_left(d):
                if d >= 0:
                    buf = pos[d % POS_BUFS]
                    t, t3, ready = buf
                    if not ready:
                        nc.scalar.copy(t[:], pos[0][0][:])
                        if d > 0:
                            zero_cols(t3, 0, d)
                        buf[2] = True
                    elif d >= POS_BUFS:
                        zero_cols(t3, d - POS_BUFS, d)
                else:
                    buf = neg[(-d - 1) % NEG_BUFS]
                    t, t3, ready = buf
                    if not ready:
                        nc.scalar.copy(t[:], pos[0][0][:])
                        zero_cols(t3, W + d, W)
                        buf[2] = True
                    else:
                        zero_cols(t3, W + d, W + d + NEG_BUFS)
                nc.gpsimd.dma_start(left_out.ap()[:, :, d - MIN_D, :, :], t[:])

            def emit_right(di):
                d = di + MIN_D
                a = PAD_L - d
                stage = stpool.tile([NPART, HL * W], f16, tag="st")
                st3 = stage[:].rearrange("p (h w) -> p h w", h=HL)
                nc.vector.tensor_copy(st3[:], rp3[:, :, a : a + W])
                nc.gpsimd.dma_start(right_out.ap()[:, :, di, :, :], stage[:])

            emit_left(LEFT_ORDER[0])
            emit_left(LEFT_ORDER[1])
            for step in range(D):
                emit_right(step)
                if step + 2 < D:
                    emit_left(LEFT_ORDER[step + 2])

    nc.compile()
    return nc


def _get_nc():
    if "nc" not in _CACHE:
        _CACHE["nc"] = _build_nc()
    return _CACHE["nc"]


def kernel(left_feat, right_feat):
    from concourse.bass_utils import run_bass_kernel_spmd

    left = np.asarray(left_feat)
    right = np.asarray(right_feat)
    assert left.shape == (B, C, H, W) and right.shape == (B, C, H, W)

    nc = _get_nc()
    left16 = left.astype(np.float16)
    right_pad16 = np.zeros((B, C, H, WP), dtype=np.float16)
    right_pad16[:, :, :, PAD_L : PAD_L + W] = right
    in_maps = []
    for m in range(N_CORES):
        rows = slice(m * HB, (m + 1) * HB)
        in_maps.append(
            {
                "left_in": np.ascontiguousarray(left16[:, :, rows, :]),
                "right_in": np.ascontiguousarray(right_pad16[:, :, rows, :]),
            }
        )
    res = run_bass_kernel_spmd(nc, in_maps, core_ids=list(range(N_CORES))).results

    out = np.empty((B, 2 * C, D, H, W), dtype=np.float32)
    for m in range(N_CORES):
        rows = slice(m * HB, (m + 1) * HB)
        out[:, :C, :, rows, :] = res[m]["left_out"]
        out[:, C:, :, rows, :] = res[m]["right_out"]
    return out


# revision 9
# speedup vs baseline: 1.8643x; 1.0396x over previous
"""Cost-volume kernel for Trainium2 (Bass/Tile), 8-core SPMD.

Problem: left/right features [B=2, C=32, H=128, W=256] f32.
Output [B, 2C=64, D=48, H, W] where for disparity d in [-8, 40):
  out[:, 0:C,  d+8, h, x] = left[:, :, h, x]   if 0 <= x-d < W else 0
  out[:, C:2C, d+8, h, x] = right[:, :, h, x-d] if 0 <= x-d < W else 0

This is a pure data-movement kernel bound by HBM write bandwidth
(~358 GB/s per core). Two levers vs the f32 baseline (298 us):
  - fp16 end-to-end: host quantizes inputs to fp16, the device moves
    fp16 (half the HBM bytes), host upcasts the output to f32. The
    quantization rel-err (~5e-4) is far inside the 2e-2 gate.
  - H-row sharding (16 rows of H per core) instead of channel
    sharding: per-core input reads drop 2x (each core reads only its
    row band of both images).

Sharding: H split 16-rows-per-core (8 cores, identical program).
Each core builds the full disparity volume for all 64 channels of its
row band. Per-core HBM traffic: 48 MiB out + ~1.1 MiB in.

Perf notes inherited from the f32 baseline (NTFF traces):
  - SWDGE (gpsimd) engages all 16 SDMA engines; HWDGE only 8. All
    big transfers go SWDGE.
  - Every store is a full-width DMA with contiguous 4 KiB/partition
    source rows. Right-side shifted windows are materialized by DVE
    copies into contiguous staging buffers.
  - Zero padding is produced in SBUF (host-padded right image, SBUF
    memsets for left), never as thin strided DRAM writes (measured
    slower at f32: 348 vs 298 us).
  - Left-buffer prep runs on ACT so WAR-gated zeroing never blocks
    the in-order DVE queue feeding the right staging copies.
"""

import numpy as np

B, C, H, W = 2, 32, 128, 256
MIN_D, MAX_D = -8, 40
D = MAX_D - MIN_D  # 48
N_CORES = 8
HB = H // N_CORES  # 16 rows of H per core

PAD_L = 39  # covers max shift d=39 (offset = x - d + PAD_L >= 0)
PAD_R = 9   # covers min shift d=-8 (x - d <= 263 -> offset 302 < 304)
WP = PAD_L + W + PAD_R  # 304

HL = 8             # h rows held per partition
HH = HB // HL      # 2
NPART = B * C * HH  # 128 partitions: p = (b*C + c)*HH + h_hi

POS_BUFS = 6  # left work buffers for d >= 0 (buffer j: d = j, j+6, ... asc)
NEG_BUFS = 2  # left work buffers for d < 0 (buffer j: d = -(j+1), -(j+1)-2, ... desc)
STAGE_BUFS = 20  # right staging rotation depth (deep: keeps SDMA queues fed)

# store order for the left side: negatives interleaved early; within a
# buffer positives ascend and negatives descend (zero regions only grow).
LEFT_ORDER = [0, -1, 1, -2, 2, 3, -3, 4, 5, -4, 6, 7, -5, 8, 9, -6, 10,
              11, -7, 12, 13, -8] + list(range(14, MAX_D))
assert sorted(LEFT_ORDER) == list(range(MIN_D, MAX_D))
# left stores are spread over the first LEFT_SPAN store slots (of 2*D)
# so the tail of the emission stream is WAR-free right stores only.
LEFT_SPAN = 84

_CACHE = {}


def _build_nc():
    import concourse.bacc as bacc
    import concourse.tile as tile
    import concourse.mybir as mybir

    f16 = mybir.dt.float16
    alu = mybir.AluOpType

    nc = bacc.Bacc(
        "TRN2",
        target_bir_lowering=False,
        debug=False,
        enable_asserts=False,
        num_devices=N_CORES,
    )
    left_in = nc.dram_tensor("left_in", [B, C, HB, W], f16, kind="ExternalInput")
    right_in = nc.dram_tensor(
        "right_in", [B, C, HB, WP], f16, kind="ExternalInput"
    )  # host-padded with zeros: data columns at [PAD_L, PAD_L + W)
    left_out = nc.dram_tensor(
        "left_out", [B, C, D, HB, W], f16, kind="ExternalOutput"
    )
    right_out = nc.dram_tensor(
        "right_out", [B, C, D, HB, W], f16, kind="ExternalOutput"
    )

    with tile.TileContext(nc) as tc:
        with (
            tc.tile_pool(name="pool", bufs=1) as pool,
            tc.tile_pool(name="stpool", bufs=STAGE_BUFS) as stpool,
        ):
            # ---- right image (pre-padded), loaded once ----
            rp = pool.tile([NPART, HL * WP], f16, tag="rp")
            rp3 = rp[:].rearrange("p (h w) -> p h w", h=HL)
            # zero source for left-edge zeroing, done as ACT copies so the
            # WAR-gated zeroing never head-of-line blocks the in-order DVE
            # queue that feeds the right-side staging copies
            zt = pool.tile([NPART, HL * max(POS_BUFS, NEG_BUFS)], f16, tag="zt")
            zt3 = zt[:].rearrange("p (h w) -> p h w", h=HL)
            nc.vector.memset(zt[:], 0.0)

            def zero_cols(t3, a, b):
                nc.scalar.copy(t3[:, :, a:b], zt3[:, :, 0 : b - a])

            # DVE copy that never enters 2-port perf mode: a plain
            # tensor_copy locks GpSimd out of the SBUF ports holding the
            # SWDGE descriptor rings for the whole instruction, which
            # stalls ALL SWDGE dma emission (measured: an 11 us stall on
            # the first right store). tensor_tensor-class ops are
            # single-port and never contend with the Q7 descriptor
            # writer, at the cost of reading the source twice.
            def tt_copy(out_ap, in_ap):
                nc.vector.scalar_tensor_tensor(
                    out_ap, in_ap, 0.0, in_ap, alu.mult, alu.add
                )

            # ---- left work buffers; pos[0] is the load target ----
            pos = []
            neg = []
            for j in range(POS_BUFS):
                t = pool.tile([NPART, HL * W], f16, tag=f"lp{j}")
                pos.append((t, t[:].rearrange("p (h w) -> p h w", h=HL)))
            for j in range(NEG_BUFS):
                t = pool.tile([NPART, HL * W], f16, tag=f"ln{j}")
                neg.append((t, t[:].rearrange("p (h w) -> p h w", h=HL)))
            # left load first (split so the d=0 store can start on the
            # first half early): the d=0 left store depends only on it.
            # Right stores additionally need a DVE staging copy after the
            # right load lands.
            nc.gpsimd.dma_start(pos[0][0][0:64, :], left_in.ap()[0:1])
            nc.gpsimd.dma_start(pos[0][0][64:128, :], left_in.ap()[1:2])
            nc.gpsimd.dma_start(rp[:], right_in.ap())

            # eager buffer prep: cheap DVE copies (~0.5 us each at fp16)
            # instead of lazy 3.4 us ACT copies that serialized the
            # in-order gpsimd queue during the ramp. Initial zero bands
            # go on ACT right after.
            prep = [neg[0], pos[1], neg[1], pos[2], pos[3], pos[4], pos[5]]
            for t, _ in prep:
                tt_copy(t[:], pos[0][0][:])
            for j in range(NEG_BUFS):
                zero_cols(neg[j][1], W - (j + 1), W)  # first serves d=-(j+1)
            for j in range(1, POS_BUFS):
                zero_cols(pos[j][1], 0, j)  # buffer j first serves d=j

            def emit_left(d):
                if d >= 0:
                    t, t3 = pos[d % POS_BUFS]
                    if d >= POS_BUFS:
                        zero_cols(t3, d - POS_BUFS, d)
                else:
                    t, t3 = neg[(-d - 1) % NEG_BUFS]
                    if -d - 1 >= NEG_BUFS:
                        zero_cols(t3, W + d, W + d + NEG_BUFS)
                if d == 0:
                    # split like the load so the first store starts ASAP
                    nc.gpsimd.dma_start(
                        left_out.ap()[0:1, :, d - MIN_D, :, :], t[0:64, :]
                    )
                    nc.gpsimd.dma_start(
                        left_out.ap()[1:2, :, d - MIN_D, :, :], t[64:128, :]
                    )
                else:
                    nc.gpsimd.dma_start(left_out.ap()[:, :, d - MIN_D, :, :], t[:])

            def emit_right(di):
                d = di + MIN_D
                a = PAD_L - d
                stage = stpool.tile([NPART, HL * W], f16, tag="st")
                st3 = stage[:].rearrange("p (h w) -> p h w", h=HL)
                tt_copy(st3[:], rp3[:, :, a : a + W])
                nc.gpsimd.dma_start(right_out.ap()[:, :, di, :, :], stage[:])

            li = ri = 0
            for slot in range(2 * D):
                due = min(len(LEFT_ORDER), 1 + slot * (len(LEFT_ORDER) - 1) // (LEFT_SPAN - 1))
                if li < due:
                    emit_left(LEFT_ORDER[li])
                    li += 1
                else:
                    emit_right(ri)
                    ri += 1
            assert li == len(LEFT_ORDER) and ri == D

    nc.compile()
    return nc


def _get_nc():
    if "nc" not in _CACHE:
        _CACHE["nc"] = _build_nc()
    return _CACHE["nc"]


def kernel(left_feat, right_feat):
    from concourse.bass_utils import run_bass_kernel_spmd

    left = np.asarray(left_feat)
    right = np.asarray(right_feat)
    assert left.shape == (B, C, H, W) and right.shape == (B, C, H, W)

    nc = _get_nc()
    left16 = left.astype(np.float16)
    right_pad16 = np.zeros((B, C, H, WP), dtype=np.float16)
    right_pad16[:, :, :, PAD_L : PAD_L + W] = right
    in_maps = []
    for m in range(N_CORES):
        rows = slice(m * HB, (m + 1) * HB)
        in_maps.append(
            {
                "left_in": np.ascontiguousarray(left16[:, :, rows, :]),
                "right_in": np.ascontiguousarray(right_pad16[:, :, rows, :]),
            }
        )
    res = run_bass_kernel_spmd(nc, in_maps, core_ids=list(range(N_CORES))).results

    out = np.empty((B, 2 * C, D, H, W), dtype=np.float32)
    for m in range(N_CORES):
        rows = slice(m * HB, (m + 1) * HB)
        out[:, :C, :, rows, :] = res[m]["left_out"]
        out[:, C:, :, rows, :] = res[m]["right_out"]
    return out


# revision 14
# speedup vs baseline: 2.1776x; 1.1680x over previous
"""Cost-volume kernel for Trainium2 (Bass/Tile), 8-core SPMD.

Problem: left/right features [B=2, C=32, H=128, W=256] f32.
Output [B, 2C=64, D=48, H, W] where for disparity d in [-8, 40):
  out[:, 0:C,  d+8, h, x] = left[:, :, h, x]   if 0 <= x-d < W else 0
  out[:, C:2C, d+8, h, x] = right[:, :, h, x-d] if 0 <= x-d < W else 0

This is a pure data-movement kernel bound by HBM write bandwidth
(~358 GB/s per core). Two levers vs the f32 baseline (298 us):
  - fp16 end-to-end: host quantizes inputs to fp16, the device moves
    fp16 (half the HBM bytes), host upcasts the output to f32. The
    quantization rel-err (~5e-4) is far inside the 2e-2 gate.
  - H-row sharding (16 rows of H per core) instead of channel
    sharding: per-core input reads drop 2x (each core reads only its
    row band of both images).

Sharding: H split 16-rows-per-core (8 cores, identical program).
Each core builds the full disparity volume for all 64 channels of its
row band. Per-core HBM traffic: 48 MiB out + ~1.1 MiB in.

Perf notes inherited from the f32 baseline (NTFF traces):
  - SWDGE (gpsimd) engages all 16 SDMA engines; HWDGE only 8. All
    big transfers go SWDGE.
  - Every store is a full-width DMA with contiguous 4 KiB/partition
    source rows. Right-side shifted windows are materialized by DVE
    copies into contiguous staging buffers.
  - Zero padding is produced in SBUF (host-padded right image, SBUF
    memsets for left), never as thin strided DRAM writes (measured
    slower at f32: 348 vs 298 us).
  - Left-buffer prep runs on ACT so WAR-gated zeroing never blocks
    the in-order DVE queue feeding the right staging copies.
"""

import numpy as np

B, C, H, W = 2, 32, 128, 256
MIN_D, MAX_D = -8, 40
D = MAX_D - MIN_D  # 48
N_CORES = 8
HB = H // N_CORES  # 16 rows of H per core

PAD_L = 39  # covers max shift d=39 (offset = x - d + PAD_L >= 0)
PAD_R = 9   # covers min shift d=-8 (x - d <= 263 -> offset 302 < 304)
WP = PAD_L + W + PAD_R  # 304

HL = 8             # h rows held per partition
HH = HB // HL      # 2
NPART = B * C * HH  # 128 partitions: p = (b*C + c)*HH + h_hi

POS_BUFS = 6  # left work buffers for d >= 0 (buffer j: d = j, j+6, ... asc)
NEG_BUFS = 2  # left work buffers for d < 0 (buffer j: d = -(j+1), -(j+1)-2, ... desc)
STAGE_BUFS = 24  # right staging rotation depth (deep: keeps SDMA queues fed)

# store order for the left side: negatives interleaved early; within a
# buffer positives ascend and negatives descend (zero regions only grow).
LEFT_ORDER = [0, -1, 1, -2, 2, 3, -3, 4, 5, -4, 6, 7, -5, 8, 9, -6, 10,
              11, -7, 12, 13, -8] + list(range(14, MAX_D))
assert sorted(LEFT_ORDER) == list(range(MIN_D, MAX_D))
# left stores are spread over the first LEFT_SPAN store slots (of 2*D)
# so the tail of the emission stream is WAR-free right stores only.
LEFT_SPAN = 84

_CACHE = {}


def _build_nc():
    import concourse.bacc as bacc
    import concourse.tile as tile
    import concourse.mybir as mybir

    f16 = mybir.dt.float16
    alu = mybir.AluOpType

    nc = bacc.Bacc(
        "TRN2",
        target_bir_lowering=False,
        debug=False,
        enable_asserts=False,
        num_devices=N_CORES,
    )
    left_in = nc.dram_tensor("left_in", [B, C, HB, W], f16, kind="ExternalInput")
    right_in = nc.dram_tensor(
        "right_in", [B, C, HB, WP], f16, kind="ExternalInput"
    )  # host-padded with zeros: data columns at [PAD_L, PAD_L + W)
    left_out = nc.dram_tensor(
        "left_out", [B, C, D, HB, W], f16, kind="ExternalOutput"
    )
    right_out = nc.dram_tensor(
        "right_out", [B, C, D, HB, W], f16, kind="ExternalOutput"
    )

    with tile.TileContext(nc) as tc:
        with (
            tc.tile_pool(name="pool", bufs=1) as pool,
            tc.tile_pool(name="stpool", bufs=STAGE_BUFS) as stpool,
        ):
            # ---- right image (pre-padded), loaded once ----
            rp = pool.tile([NPART, HL * WP], f16, tag="rp")
            rp3 = rp[:].rearrange("p (h w) -> p h w", h=HL)
            # zero source for left-edge zeroing, done as ACT copies so the
            # WAR-gated zeroing never head-of-line blocks the in-order DVE
            # queue that feeds the right-side staging copies
            zt = pool.tile([NPART, HL * max(POS_BUFS, NEG_BUFS)], f16, tag="zt")
            zt3 = zt[:].rearrange("p (h w) -> p h w", h=HL)
            nc.vector.memset(zt[:], 0.0)

            def zero_cols(t3, a, b):
                nc.scalar.copy(t3[:, :, a:b], zt3[:, :, 0 : b - a])



            # ---- left work buffers; pos[0] is the load target ----
            pos = []
            neg = []
            for j in range(POS_BUFS):
                t = pool.tile([NPART, HL * W], f16, tag=f"lp{j}")
                pos.append((t, t[:].rearrange("p (h w) -> p h w", h=HL)))
            for j in range(NEG_BUFS):
                t = pool.tile([NPART, HL * W], f16, tag=f"ln{j}")
                neg.append((t, t[:].rearrange("p (h w) -> p h w", h=HL)))
            # left load first (split so the d=0 store can start on the
            # first half early): the d=0 left store depends only on it.
            # Right stores additionally need a DVE staging copy after the
            # right load lands.
            nc.gpsimd.dma_start(pos[0][0][0:64, :], left_in.ap()[0:1])
            nc.gpsimd.dma_start(pos[0][0][64:128, :], left_in.ap()[1:2])
            nc.gpsimd.dma_start(rp[:], right_in.ap())

            # eager buffer prep: cheap DVE copies (~0.5 us each at fp16)
            # instead of lazy 3.4 us ACT copies that serialized the
            # in-order gpsimd queue during the ramp. Initial zero bands
            # go on ACT right after.
            prep = [neg[0], pos[1], neg[1], pos[2], pos[3], pos[4], pos[5]]
            for t, _ in prep:
                nc.vector.tensor_copy(t[:], pos[0][0][:])
            for j in range(NEG_BUFS):
                zero_cols(neg[j][1], W - (j + 1), W)  # first serves d=-(j+1)
            for j in range(1, POS_BUFS):
                zero_cols(pos[j][1], 0, j)  # buffer j first serves d=j

            def emit_left(d):
                if d >= 0:
                    t, t3 = pos[d % POS_BUFS]
                    if d >= POS_BUFS:
                        zero_cols(t3, d - POS_BUFS, d)
                else:
                    t, t3 = neg[(-d - 1) % NEG_BUFS]
                    if -d - 1 >= NEG_BUFS:
                        zero_cols(t3, W + d, W + d + NEG_BUFS)
                if d == 0:
                    # split like the load so the first store starts ASAP
                    nc.scalar.dma_start(
                        left_out.ap()[0:1, :, d - MIN_D, :, :], t[0:64, :]
                    )
                    nc.scalar.dma_start(
                        left_out.ap()[1:2, :, d - MIN_D, :, :], t[64:128, :]
                    )
                else:
                    nc.scalar.dma_start(left_out.ap()[:, :, d - MIN_D, :, :], t[:])

            def emit_right(di):
                d = di + MIN_D
                a = PAD_L - d
                stage = stpool.tile([NPART, HL * W], f16, tag="st")
                st3 = stage[:].rearrange("p (h w) -> p h w", h=HL)
                nc.vector.tensor_copy(st3[:], rp3[:, :, a : a + W])
                nc.sync.dma_start(right_out.ap()[:, :, di, :, :], stage[:])

            li = ri = 0
            for slot in range(2 * D):
                due = min(len(LEFT_ORDER), 1 + slot * (len(LEFT_ORDER) - 1) // (LEFT_SPAN - 1))
                if li < due:
                    emit_left(LEFT_ORDER[li])
                    li += 1
                else:
                    emit_right(ri)
                    ri += 1
            assert li == len(LEFT_ORDER) and ri == D

    nc.compile()
    return nc


def _get_nc():
    if "nc" not in _CACHE:
        _CACHE["nc"] = _build_nc()
    return _CACHE["nc"]


def kernel(left_feat, right_feat):
    from concourse.bass_utils import run_bass_kernel_spmd

    left = np.asarray(left_feat)
    right = np.asarray(right_feat)
    assert left.shape == (B, C, H, W) and right.shape == (B, C, H, W)

    nc = _get_nc()
    left16 = left.astype(np.float16)
    right_pad16 = np.zeros((B, C, H, WP), dtype=np.float16)
    right_pad16[:, :, :, PAD_L : PAD_L + W] = right
    in_maps = []
    for m in range(N_CORES):
        rows = slice(m * HB, (m + 1) * HB)
        in_maps.append(
            {
                "left_in": np.ascontiguousarray(left16[:, :, rows, :]),
                "right_in": np.ascontiguousarray(right_pad16[:, :, rows, :]),
            }
        )
    res = run_bass_kernel_spmd(nc, in_maps, core_ids=list(range(N_CORES))).results

    out = np.empty((B, 2 * C, D, H, W), dtype=np.float32)
    for m in range(N_CORES):
        rows = slice(m * HB, (m + 1) * HB)
        out[:, :C, :, rows, :] = res[m]["left_out"]
        out[:, C:, :, rows, :] = res[m]["right_out"]
    return out


# revision 18
# speedup vs baseline: 2.1876x; 1.0046x over previous
"""Cost-volume kernel for Trainium2 (Bass/Tile), 8-core SPMD.

Problem: left/right features [B=2, C=32, H=128, W=256] f32.
Output [B, 2C=64, D=48, H, W] where for disparity d in [-8, 40):
  out[:, 0:C,  d+8, h, x] = left[:, :, h, x]   if 0 <= x-d < W else 0
  out[:, C:2C, d+8, h, x] = right[:, :, h, x-d] if 0 <= x-d < W else 0

This is a pure data-movement kernel bound by HBM write bandwidth
(~358 GB/s per core). Two levers vs the f32 baseline (298 us):
  - fp16 end-to-end: host quantizes inputs to fp16, the device moves
    fp16 (half the HBM bytes), host upcasts the output to f32. The
    quantization rel-err (~5e-4) is far inside the 2e-2 gate.
  - H-row sharding (16 rows of H per core) instead of channel
    sharding: per-core input reads drop 2x (each core reads only its
    row band of both images).

Sharding: H split 16-rows-per-core (8 cores, identical program).
Each core builds the full disparity volume for all 64 channels of its
row band. Per-core HBM traffic: 48 MiB out + ~1.1 MiB in.

Perf notes inherited from the f32 baseline (NTFF traces):
  - SWDGE (gpsimd) engages all 16 SDMA engines; HWDGE only 8. All
    big transfers go SWDGE.
  - Every store is a full-width DMA with contiguous 4 KiB/partition
    source rows. Right-side shifted windows are materialized by DVE
    copies into contiguous staging buffers.
  - Zero padding is produced in SBUF (host-padded right image, SBUF
    memsets for left), never as thin strided DRAM writes (measured
    slower at f32: 348 vs 298 us).
  - Left-buffer prep runs on ACT so WAR-gated zeroing never blocks
    the in-order DVE queue feeding the right staging copies.
"""

import numpy as np

B, C, H, W = 2, 32, 128, 256
MIN_D, MAX_D = -8, 40
D = MAX_D - MIN_D  # 48
N_CORES = 8
HB = H // N_CORES  # 16 rows of H per core

PAD_L = 39  # covers max shift d=39 (offset = x - d + PAD_L >= 0)
PAD_R = 9   # covers min shift d=-8 (x - d <= 263 -> offset 302 < 304)
WP = PAD_L + W + PAD_R  # 304

HL = 8             # h rows held per partition
HH = HB // HL      # 2
NPART = B * C * HH  # 128 partitions: p = (b*C + c)*HH + h_hi

POS_BUFS = 12  # left work buffers for d >= 0 (buffer j: d = j, j+12, ... asc)
NEG_BUFS = 2  # left work buffers for d < 0 (buffer j: d = -(j+1), -(j+1)-2, ... desc)
STAGE_BUFS = 24  # right staging rotation depth (deep: keeps SDMA queues fed)
PRESTAGE = 8  # right staging copies interleaved with buffer prep on DVE

# store order for the left side: negatives interleaved early; within a
# buffer positives ascend and negatives descend (zero regions only grow).
LEFT_ORDER = [0, -1, 1, -2, 2, 3, -3, 4, 5, -4, 6, 7, -5, 8, 9, -6, 10,
              11, -7, 12, 13, -8] + list(range(14, MAX_D))
assert sorted(LEFT_ORDER) == list(range(MIN_D, MAX_D))
# left stores are spread over the first LEFT_SPAN store slots (of 2*D)
# so the tail of the emission stream is WAR-free right stores only.
LEFT_SPAN = 84

_CACHE = {}


def _build_nc():
    import concourse.bacc as bacc
    import concourse.tile as tile
    import concourse.mybir as mybir

    f16 = mybir.dt.float16
    alu = mybir.AluOpType

    nc = bacc.Bacc(
        "TRN2",
        target_bir_lowering=False,
        debug=False,
        enable_asserts=False,
        num_devices=N_CORES,
    )
    left_in = nc.dram_tensor("left_in", [B, C, HB, W], f16, kind="ExternalInput")
    right_in = nc.dram_tensor(
        "right_in", [B, C, HB, WP], f16, kind="ExternalInput"
    )  # host-padded with zeros: data columns at [PAD_L, PAD_L + W)
    left_out = nc.dram_tensor(
        "left_out", [B, C, D, HB, W], f16, kind="ExternalOutput"
    )
    right_out = nc.dram_tensor(
        "right_out", [B, C, D, HB, W], f16, kind="ExternalOutput"
    )

    with tile.TileContext(nc) as tc:
        with (
            tc.tile_pool(name="pool", bufs=1) as pool,
            tc.tile_pool(name="stpool", bufs=STAGE_BUFS) as stpool,
        ):
            # ---- right image (pre-padded), loaded once ----
            rp = pool.tile([NPART, HL * WP], f16, tag="rp")
            rp3 = rp[:].rearrange("p (h w) -> p h w", h=HL)
            # zero source for left-edge zeroing, done as ACT copies so the
            # WAR-gated zeroing never head-of-line blocks the in-order DVE
            # queue that feeds the right-side staging copies
            zt = pool.tile([NPART, HL * max(POS_BUFS, NEG_BUFS)], f16, tag="zt")
            zt3 = zt[:].rearrange("p (h w) -> p h w", h=HL)
            nc.vector.memset(zt[:], 0.0)

            def zero_cols(t3, a, b):
                nc.scalar.copy(t3[:, :, a:b], zt3[:, :, 0 : b - a])


            # ---- left work buffers; pos[0] is the load target ----
            pos = []
            neg = []
            for j in range(POS_BUFS):
                t = pool.tile([NPART, HL * W], f16, tag=f"lp{j}")
                pos.append((t, t[:].rearrange("p (h w) -> p h w", h=HL)))
            for j in range(NEG_BUFS):
                t = pool.tile([NPART, HL * W], f16, tag=f"ln{j}")
                neg.append((t, t[:].rearrange("p (h w) -> p h w", h=HL)))
            # left load first (split so the d=0 store can start on the
            # first half early): the d=0 left store depends only on it.
            # Right stores additionally need a DVE staging copy after the
            # right load lands.
            nc.gpsimd.dma_start(pos[0][0][0:64, :], left_in.ap()[0:1])
            nc.gpsimd.dma_start(pos[0][0][64:128, :], left_in.ap()[1:2])
            nc.gpsimd.dma_start(rp[:], right_in.ap())

            # eager buffer prep: cheap DVE copies (~0.5 us each at fp16)
            # instead of lazy 3.4 us ACT copies that serialized the
            # in-order gpsimd queue during the ramp. Initial zero bands
            # go on ACT right after.
            # staging copies for the first rights, interleaved with the
            # buffer-prep copies in the DVE queue so the right stores
            # (Sync HWDGE ring) start flowing at ~12 us instead of
            # waiting for every prep to finish first.
            def make_stage(di):
                d = di + MIN_D
                a = PAD_L - d
                stage = stpool.tile([NPART, HL * W], f16, tag="st")
                st3 = stage[:].rearrange("p (h w) -> p h w", h=HL)
                nc.vector.tensor_copy(st3[:], rp3[:, :, a : a + W])
                return stage

            prep = [neg[0], pos[1], neg[1]] + [pos[j] for j in range(2, POS_BUFS)]
            prestaged = {}
            for k in range(max(len(prep), PRESTAGE)):
                if k < PRESTAGE:
                    prestaged[k] = make_stage(k)
                if k < len(prep):
                    nc.vector.tensor_copy(prep[k][0][:], pos[0][0][:])
            for j in range(NEG_BUFS):
                zero_cols(neg[j][1], W - (j + 1), W)  # first serves d=-(j+1)
            for j in range(1, POS_BUFS):
                zero_cols(pos[j][1], 0, j)  # buffer j first serves d=j

            def emit_left(d):
                if d >= 0:
                    t, t3 = pos[d % POS_BUFS]
                    if d >= POS_BUFS:
                        zero_cols(t3, d - POS_BUFS, d)
                else:
                    t, t3 = neg[(-d - 1) % NEG_BUFS]
                    if -d - 1 >= NEG_BUFS:
                        zero_cols(t3, W + d, W + d + NEG_BUFS)
                if d == 0:
                    # split like the load so the first store starts ASAP
                    nc.scalar.dma_start(
                        left_out.ap()[0:1, :, d - MIN_D, :, :], t[0:64, :]
                    )
                    nc.scalar.dma_start(
                        left_out.ap()[1:2, :, d - MIN_D, :, :], t[64:128, :]
                    )
                else:
                    nc.scalar.dma_start(left_out.ap()[:, :, d - MIN_D, :, :], t[:])

            def emit_right(di):
                stage = prestaged.pop(di, None)
                if stage is None:
                    stage = make_stage(di)
                nc.sync.dma_start(right_out.ap()[:, :, di, :, :], stage[:])

            li = ri = 0
            for slot in range(2 * D):
                due = min(len(LEFT_ORDER), 1 + slot * (len(LEFT_ORDER) - 1) // (LEFT_SPAN - 1))
                if li < due:
                    emit_left(LEFT_ORDER[li])
                    li += 1
                else:
                    emit_right(ri)
                    ri += 1
            assert li == len(LEFT_ORDER) and ri == D

    nc.compile()
    return nc


def _get_nc():
    if "nc" not in _CACHE:
        _CACHE["nc"] = _build_nc()
    return _CACHE["nc"]


def kernel(left_feat, right_feat):
    from concourse.bass_utils import run_bass_kernel_spmd

    left = np.asarray(left_feat)
    right = np.asarray(right_feat)
    assert left.shape == (B, C, H, W) and right.shape == (B, C, H, W)

    nc = _get_nc()
    left16 = left.astype(np.float16)
    right_pad16 = np.zeros((B, C, H, WP), dtype=np.float16)
    right_pad16[:, :, :, PAD_L : PAD_L + W] = right
    in_maps = []
    for m in range(N_CORES):
        rows = slice(m * HB, (m + 1) * HB)
        in_maps.append(
            {
                "left_in": np.ascontiguousarray(left16[:, :, rows, :]),
                "right_in": np.ascontiguousarray(right_pad16[:, :, rows, :]),
            }
        )
    res = run_bass_kernel_spmd(nc, in_maps, core_ids=list(range(N_CORES))).results

    out = np.empty((B, 2 * C, D, H, W), dtype=np.float32)
    for m in range(N_CORES):
        rows = slice(m * HB, (m + 1) * HB)
        out[:, :C, :, rows, :] = res[m]["left_out"]
        out[:, C:, :, rows, :] = res[m]["right_out"]
    return out

